# revision 1
# baseline (speedup 1.0000x reference)
"""Trainium2 Bass kernel for nn_GameTensor_27195732918735.

Computes out[i,j,b] = Hessian_z V_i(z_all[j,b]) for i != j, zeros on the
diagonal, where V_i(z) = W2[i] @ tanh(W1[i] @ z + b1[i]) + b2[i].

Analytic form used on-device:
    u = W1 z + b1;  th = tanh(u);  s_k = -2 W2_k th_k (1 - th_k^2)
    H = W1^T diag(s) W1  =  sum_k s_k w1_k w1_k^T

Per-core plan (8 cores, SPMD):
  core c owns agent i = c//2 and three (j, batch-half) "tasks" (the 12
  nonzero (i,j) cells x 2 batch halves = 24 half-cells / 8 cores = 3).
  On-chip: T[k, a*128+c] = W1[k,a] * W1[k,c] is precomputed once per core
  (agent-local), then each task's Hessians for its 128 batches are a single
  [k=256] x [b=128] x [(a,c)=16384] matmul H[b,(a,c)] = sum_k S[k,b] T[k,(a,c)]
  with perfectly contiguous output DMA. Diagonal zero blocks are written
  host-side (they are constants).
"""

import numpy as np

import concourse.bass as bass
import concourse.mybir as mybir
import concourse.tile as tile
from concourse import bacc
from concourse.bass_utils import run_bass_kernel_spmd

N, B, D = 4, 256, 128
H2 = 2 * D  # 256 hidden
NCORES = 8
NTASK = 3  # (j, half) tasks per core
HALF = B // 2  # 128 batches per task

# matmul operand dtype for the big S^T @ T matmuls:
#   "bf16"  : bfloat16 operands (1 cyc/row, ~0.3% rel err)
#   "fp16"  : float16 operands (1 cyc/row, ~5e-4 rel err, 2x DVE T-precompute)
#   "f32r"  : float32r operands (4-byte, 1 cyc/row at N>=512 per cost model)
#   "f32"   : plain float32 (4 cyc/row, exact)
MM_MODE = "f32r"

_F32 = mybir.dt.float32


def _mm_store_dtype():
    if MM_MODE == "bf16":
        return mybir.dt.bfloat16
    if MM_MODE == "fp16":
        return mybir.dt.float16
    if MM_MODE == "f32r":
        return mybir.dt.float32r
    return _F32


def _mm_view(ap):
    return ap


def _emit(tc, nc, w1c, w1t, b1c, w2s, zt, out):
    mmdt = _mm_store_dtype()
    Tanh = mybir.ActivationFunctionType.Tanh
    mult = mybir.AluOpType.mult
    add = mybir.AluOpType.add

    with (
        tc.tile_pool(name="consts", bufs=1) as consts,
        tc.tile_pool(name="tpool", bufs=1) as tpool,
        tc.tile_pool(name="small", bufs=4) as small,
        tc.tile_pool(name="stage", bufs=3) as stage_pool,
        tc.tile_pool(name="upsum", bufs=2, space="PSUM") as upsum,
        tc.tile_pool(name="psum", bufs=6, space="PSUM") as psum,
    ):
        # ---- load constants -------------------------------------------------
        w1c_sb = consts.tile([128, 2, 128], _F32)  # [k%128, kchunk, c]
        nc.sync.dma_start(w1c_sb, w1c)
        w1t_sb = consts.tile([128, 256], _F32)  # [d, k]
        nc.sync.dma_start(w1t_sb, w1t)
        b1_sb = consts.tile([128, 2], _F32)
        nc.sync.dma_start(b1_sb, b1c)
        w2s_sb = consts.tile([128, 2], _F32)  # -2*W2, [k%128, kchunk]
        nc.sync.dma_start(w2s_sb, w2s)
        zt_sb = consts.tile([128, NTASK, 128], _F32)  # [d, task, b]
        nc.sync.dma_start(zt_sb, zt.rearrange("t d b -> d t b"))

        if mmdt == mybir.dt.bfloat16:
            w1m = consts.tile([128, 2, 128], mmdt)
            nc.vector.tensor_copy(out=w1m, in_=w1c_sb)
        else:
            w1m = w1c_sb

        # ---- S[k, b] per task: s = -2*W2 * th * (1 - th^2) ------------------
        s_sb = consts.tile([128, NTASK, 2, 128], mmdt)  # [k%128, task, kchunk, b]
        for t in range(NTASK):
            for kc in range(2):
                ups = upsum.tile([128, 128], _F32)
                nc.tensor.matmul(
                    ups,
                    lhsT=w1t_sb[:, kc * 128 : (kc + 1) * 128],
                    rhs=zt_sb[:, t, :],
                    start=True,
                    stop=True,
                )
                th = small.tile([128, 128], _F32, tag="th")
                nc.scalar.activation(th, ups, Tanh, bias=b1_sb[:, kc : kc + 1])
                sq = small.tile([128, 128], _F32, tag="sq")
                nc.vector.tensor_tensor(sq, th, th, mult)
                nc.vector.tensor_scalar(sq, sq, -1.0, 1.0, mult, add)
                nc.vector.tensor_tensor(sq, th, sq, mult)
                nc.vector.tensor_scalar(
                    s_sb[:, t, kc, :], sq, w2s_sb[:, kc : kc + 1], None, mult
                )

        # ---- T[k, a*128+c] = W1[k,a] * W1[k,c], 8 a-values per DVE op -------
        AG = 8  # a-values per op
        TT = tpool.tile([128, 2, 16384], mmdt)
        for g in range(128 // AG):
            for kc in range(2):
                dst = TT[:, kc, g * AG * 128 : (g + 1) * AG * 128].rearrange(
                    "p (x y) -> p x y", x=AG
                )
                in0 = w1m[:, kc, None, :].to_broadcast((128, AG, 128))
                in1 = w1m[:, kc, g * AG : (g + 1) * AG, None].to_broadcast(
                    (128, AG, 128)
                )
                nc.vector.tensor_tensor(dst, in0, in1, mult)

        # ---- main: H[b, (a,c)] = sum_k S[k,b] T[k,(a,c)] --------------------
        out_flat = [out[t].rearrange("b a c -> b (a c)") for t in range(NTASK)]
        for t in range(NTASK):
            for g4 in range(8):  # 4 n-tiles of 512 -> one 1 MiB DMA
                stg = stage_pool.tile([128, 2048], _F32)
                for nn in range(4):
                    n = g4 * 4 + nn
                    ps = psum.tile([128, 512], _F32)
                    nc.tensor.matmul(
                        ps,
                        lhsT=_mm_view(s_sb[:, t, 0, :]),
                        rhs=_mm_view(TT[:, 0, n * 512 : (n + 1) * 512]),
                        start=True,
                        stop=False,
                    )
                    nc.tensor.matmul(
                        ps,
                        lhsT=_mm_view(s_sb[:, t, 1, :]),
                        rhs=_mm_view(TT[:, 1, n * 512 : (n + 1) * 512]),
                        start=False,
                        stop=True,
                    )
                    dst = stg[:, nn * 512 : (nn + 1) * 512]
                    if n % 3 == 2:
                        nc.scalar.copy(dst, ps)
                    else:
                        nc.vector.tensor_copy(out=dst, in_=ps)
                nc.sync.dma_start(out_flat[t][:, g4 * 2048 : (g4 + 1) * 2048], stg)


_NC_CACHE = {}


def _core_tasks(c):
    i = c // 2
    js = [j for j in range(N) if j != i]
    halves = [(j, h) for j in js for h in (0, 1)]
    return i, (halves[0:3] if c % 2 == 0 else halves[3:6])


def _build():
    key = MM_MODE
    if key in _NC_CACHE:
        return _NC_CACHE[key]
    nc = bacc.Bacc("TRN2", target_bir_lowering=False, debug=False, num_devices=NCORES)
    w1c = nc.dram_tensor("w1c", [128, 2, 128], _F32, kind="ExternalInput").ap()
    w1t = nc.dram_tensor("w1t", [128, 256], _F32, kind="ExternalInput").ap()
    b1c = nc.dram_tensor("b1c", [128, 2], _F32, kind="ExternalInput").ap()
    w2s = nc.dram_tensor("w2s", [128, 2], _F32, kind="ExternalInput").ap()
    zt = nc.dram_tensor("zt", [NTASK, 128, 128], _F32, kind="ExternalInput").ap()
    out = nc.dram_tensor("out", [NTASK, HALF, D, D], _F32, kind="ExternalOutput").ap()
    with tile.TileContext(nc) as tc:
        _emit(tc, nc, w1c, w1t, b1c, w2s, zt, out)
    nc.compile()
    _NC_CACHE[key] = nc
    return nc


# Options for test harness introspection (set by test.py, unused in grading).
_RUN_KWARGS = {}
_LAST_RESULT = None


def kernel(z_all, W1, b1, W2, b2):
    global _LAST_RESULT
    z_all = np.asarray(z_all, dtype=np.float32)
    W1 = np.asarray(W1, dtype=np.float32)
    b1 = np.asarray(b1, dtype=np.float32)
    W2 = np.asarray(W2, dtype=np.float32)

    nc = _build()

    in_maps = []
    metas = []
    for c in range(NCORES):
        i, tasks = _core_tasks(c)
        metas.append((i, tasks))
        w1i = W1[i]  # [256, 128]
        in_maps.append(
            {
                "w1c": np.ascontiguousarray(
                    w1i.reshape(2, 128, 128).transpose(1, 0, 2)
                ),
                "w1t": np.ascontiguousarray(w1i.T),
                "b1c": np.ascontiguousarray(b1[i].reshape(2, 128).T),
                "w2s": np.ascontiguousarray((-2.0 * W2[i, 0]).reshape(2, 128).T),
                "zt": np.ascontiguousarray(
                    np.stack(
                        [
                            z_all[j, h * HALF : (h + 1) * HALF, :].T
                            for (j, h) in tasks
                        ]
                    )
                ),
            }
        )

    res = run_bass_kernel_spmd(nc, in_maps, list(range(NCORES)), **_RUN_KWARGS)
    _LAST_RESULT = res

    full = np.zeros((N, N, B, D, D), dtype=np.float32)
    for c in range(NCORES):
        i, tasks = metas[c]
        o = res.results[c]["out"]  # [NTASK, HALF, D, D]
        for t, (j, h) in enumerate(tasks):
            full[i, j, h * HALF : (h + 1) * HALF] = o[t]
    return full



# revision 4
# speedup vs baseline: 2.5046x; 2.5046x over previous
"""Trainium2 Bass kernel for nn_GameTensor_27195732918735.

Computes out[i,j,b] = Hessian_z V_i(z_all[j,b]) for i != j, zeros on the
diagonal, where V_i(z) = W2[i] @ tanh(W1[i] @ z + b1[i]) + b2[i].

Analytic form used on-device:
    u = W1 z + b1;  th = tanh(u);  s_k = -2 W2_k th_k (1 - th_k^2)
    H = W1^T diag(s) W1  =  sum_k s_k w1_k w1_k^T

H is symmetric in (d1, d2), so the device only computes a block-triangular
packed half: with 8-wide d1 blocks (r = d1//8), block r covers d1 in
[8r, 8r+8) x d2 in [8r, 128) -> 8704 of 16384 columns.  The host mirrors
the missing (d1, d2) entries from (d2, d1) with a precomputed gather map.

Per-core plan (8 cores, SPMD, identical program):
  core c owns agent i = c//2 and three (j, batch-half) "tasks" (12 nonzero
  (i,j) cells x 2 batch halves = 24 half-cells / 8 cores = 3).
  On-chip: T[k, col] = W1[k,d1(col)] * W1[k,d2(col)] for the packed
  triangle is built by DVE broadcast multiplies (one op per (kc, r) block);
  each task's Hessians are then 17 N=512 matmul pairs
  H[b, col] = sum_k S[k,b] T[k,col] accumulating over the two 128-row k
  chunks, evacuated from PSUM as fp16 (Scalar engine early, while the DVE
  is still building T; alternating Vector/Scalar later), and DMA'd out in
  2048-column groups.  Everything 16-bit except PSUM and bias.
"""

import numpy as np

import concourse.bass as bass
import concourse.mybir as mybir
import concourse.tile as tile
from concourse import bacc
from concourse.bass_utils import run_bass_kernel_spmd

N, B, D = 4, 256, 128
H2 = 2 * D  # 256 hidden
NCORES = 8
NTASK = 3  # (j, half) tasks per core
HALF = B // 2  # 128 batches per task

BLK = 8  # d1 block width of the packed triangle
NBLK = D // BLK  # 16
BLK_W = [D - BLK * r for r in range(NBLK)]  # d2 run width per block
BLK_OFF = [0]
for r in range(NBLK):
    BLK_OFF.append(BLK_OFF[-1] + BLK * BLK_W[r])
PACKED = BLK_OFF[-1]  # 8704
NCHUNK = PACKED // 512  # 17

# ---- tuning knobs ----------------------------------------------------------
# chunks with global index < EARLY_S are evacuated by ScalarE (VectorE is
# busy building T); afterwards even/odd indices alternate Vector/Scalar.
EARLY_S = 30
# r-blocks whose T build runs on GpSimd instead of VectorE (offload).
G_BLOCKS = ()

_F32 = mybir.dt.float32
_F16 = mybir.dt.float16


def _emit(tc, nc, w1m, w1t, b1c, w2s, zt, out):
    Tanh = mybir.ActivationFunctionType.Tanh
    Square = mybir.ActivationFunctionType.Square
    mult = mybir.AluOpType.mult
    subtract = mybir.AluOpType.subtract

    with (
        tc.tile_pool(name="consts", bufs=1) as consts,
        tc.tile_pool(name="tpool", bufs=1) as tpool,
        tc.tile_pool(name="small", bufs=1) as small,
        tc.tile_pool(name="stage", bufs=2) as stage_pool,
        tc.tile_pool(name="upsum", bufs=2, space="PSUM") as upsum,
        tc.tile_pool(name="psum", bufs=6, space="PSUM") as psum,
    ):
        # ---- load constants -------------------------------------------------
        w1m_sb = consts.tile([128, 2, 128], _F16)  # [k%128, kc, d]
        nc.sync.dma_start(w1m_sb, w1m)
        w1t_sb = consts.tile([128, 256], _F16)  # [d, k]
        nc.sync.dma_start(w1t_sb, w1t)
        b1_sb = consts.tile([128, 2], _F32)  # [k%128, kc]
        nc.sync.dma_start(b1_sb, b1c)
        w2s_sb = consts.tile([128, 2], _F32)  # -2*W2, [k%128, kc]
        nc.sync.dma_start(w2s_sb, w2s)
        zt_sb = consts.tile([128, NTASK, 128], _F16)  # [d, task, b]
        nc.sync.dma_start(zt_sb, zt)

        # ---- S[k, (t,b)] = -2*W2 * th * (1 - th^2),  th = tanh(W1 z + b1) --
        th = consts.tile([128, 2, NTASK * 128], _F16)
        sq = consts.tile([128, 2, NTASK * 128], _F16)
        t3 = small.tile([128, 2, NTASK * 128], _F16)
        s_sb = consts.tile([128, 2, NTASK * 128], _F16)
        zflat = zt_sb.rearrange("d t b -> d (t b)")
        for kc in range(2):
            ups = upsum.tile([128, NTASK * 128], _F32, tag="ups")
            nc.tensor.matmul(
                ups,
                lhsT=w1t_sb[:, kc * 128 : (kc + 1) * 128],
                rhs=zflat,
                start=True,
                stop=True,
            )
            nc.scalar.activation(th[:, kc, :], ups, Tanh, bias=b1_sb[:, kc : kc + 1])
            nc.scalar.activation(sq[:, kc, :], th[:, kc, :], Square)
            nc.vector.tensor_tensor(t3[:, kc, :], th[:, kc, :], sq[:, kc, :], mult)
            nc.vector.tensor_tensor(t3[:, kc, :], th[:, kc, :], t3[:, kc, :], subtract)
            nc.vector.tensor_scalar(
                s_sb[:, kc, :], t3[:, kc, :], w2s_sb[:, kc : kc + 1], None, mult
            )

        # ---- T[k, col] = W1[k,d1(col)] * W1[k,d2(col)], packed triangle ----
        # one broadcast multiply per (r-block, kc): [128, BLK, W_r]
        TT = tpool.tile([128, 2, PACKED], _F16)

        def emit_tblock(r):
            w = BLK_W[r]
            for kc in range(2):
                dst = TT[:, kc, BLK_OFF[r] : BLK_OFF[r + 1]].rearrange(
                    "p (x y) -> p x y", x=BLK
                )
                in0 = w1m_sb[:, kc, None, BLK * r : 128].to_broadcast((128, BLK, w))
                in1 = w1m_sb[:, kc, BLK * r : BLK * r + BLK, None].to_broadcast(
                    (128, BLK, w)
                )
                eng = nc.gpsimd if r in G_BLOCKS else nc.vector
                eng.tensor_tensor(dst, in0, in1, mult)

        for r in range(NBLK):
            emit_tblock(r)

        # ---- main: H[b, col] = sum_k S[k,(t,b)] T[k,col] -------------------
        # chunk n covers packed cols [512n, 512(n+1)); stage 4 chunks per DMA
        stg = [None] * NTASK
        idx = 0
        for n in range(NCHUNK):
            for t in range(NTASK):
                if n % 4 == 0:
                    stg[t] = stage_pool.tile(
                        [128, 2048], _F16, tag=f"stg{t}", name=f"stg{t}_{n}"
                    )
                ps = psum.tile([128, 512], _F32, tag="mm")
                nc.tensor.matmul(
                    ps,
                    lhsT=s_sb[:, 0, t * 128 : (t + 1) * 128],
                    rhs=TT[:, 0, n * 512 : (n + 1) * 512],
                    start=True,
                    stop=False,
                )
                nc.tensor.matmul(
                    ps,
                    lhsT=s_sb[:, 1, t * 128 : (t + 1) * 128],
                    rhs=TT[:, 1, n * 512 : (n + 1) * 512],
                    start=False,
                    stop=True,
                )
                dst = stg[t][:, (n % 4) * 512 : (n % 4 + 1) * 512]
                use_scalar = idx < EARLY_S or (idx - EARLY_S) % 2 == 1
                if use_scalar:
                    nc.scalar.copy(dst, ps)
                else:
                    nc.vector.tensor_copy(out=dst, in_=ps)
                idx += 1
                if n % 4 == 3 or n == NCHUNK - 1:
                    g = n // 4
                    width = (n % 4 + 1) * 512
                    nc.sync.dma_start(
                        out[t][:, g * 2048 : g * 2048 + width], stg[t][:, :width]
                    )


_NC_CACHE = {}


def _core_tasks(c):
    i = c // 2
    js = [j for j in range(N) if j != i]
    halves = [(j, h) for j in js for h in (0, 1)]
    return i, (halves[0:3] if c % 2 == 0 else halves[3:6])


def _build():
    key = (EARLY_S, tuple(G_BLOCKS))
    if key in _NC_CACHE:
        return _NC_CACHE[key]
    nc = bacc.Bacc("TRN2", target_bir_lowering=False, debug=False, num_devices=NCORES)
    w1m = nc.dram_tensor("w1m", [128, 2, 128], _F16, kind="ExternalInput").ap()
    w1t = nc.dram_tensor("w1t", [128, 256], _F16, kind="ExternalInput").ap()
    b1c = nc.dram_tensor("b1c", [128, 2], _F32, kind="ExternalInput").ap()
    w2s = nc.dram_tensor("w2s", [128, 2], _F32, kind="ExternalInput").ap()
    zt = nc.dram_tensor("zt", [128, NTASK, 128], _F16, kind="ExternalInput").ap()
    out = nc.dram_tensor("out", [NTASK, HALF, PACKED], _F16, kind="ExternalOutput").ap()
    with tile.TileContext(nc) as tc:
        _emit(tc, nc, w1m, w1t, b1c, w2s, zt, out)
    nc.compile()
    _NC_CACHE[key] = nc
    return nc


def _unpack_idx():
    # packed column of (d1, d2): stored if d2 >= 8*(d1//8), else mirror (d2, d1)
    idx = np.empty((D, D), dtype=np.int64)
    for d1 in range(D):
        r = d1 // BLK
        for d2 in range(D):
            if d2 >= BLK * r:
                idx[d1, d2] = BLK_OFF[r] + (d1 - BLK * r) * BLK_W[r] + (d2 - BLK * r)
            else:
                r2 = d2 // BLK
                idx[d1, d2] = BLK_OFF[r2] + (d2 - BLK * r2) * BLK_W[r2] + (d1 - BLK * r2)
    return idx.reshape(-1)


_UNPACK_IDX = None


# Options for test harness introspection (set by test.py, unused in grading).
_RUN_KWARGS = {}
_LAST_RESULT = None


def kernel(z_all, W1, b1, W2, b2):
    global _LAST_RESULT, _UNPACK_IDX
    z_all = np.asarray(z_all, dtype=np.float32)
    W1 = np.asarray(W1, dtype=np.float32)
    b1 = np.asarray(b1, dtype=np.float32)
    W2 = np.asarray(W2, dtype=np.float32)

    nc = _build()
    if _UNPACK_IDX is None:
        _UNPACK_IDX = _unpack_idx()

    in_maps = []
    metas = []
    for c in range(NCORES):
        i, tasks = _core_tasks(c)
        metas.append((i, tasks))
        w1i = W1[i]  # [256, 128]
        in_maps.append(
            {
                "w1m": np.ascontiguousarray(
                    w1i.reshape(2, 128, 128).transpose(1, 0, 2)
                ).astype(np.float16),
                "w1t": np.ascontiguousarray(w1i.T).astype(np.float16),
                "b1c": np.ascontiguousarray(b1[i].reshape(2, 128).T),
                "w2s": np.ascontiguousarray((-2.0 * W2[i, 0]).reshape(2, 128).T),
                "zt": np.ascontiguousarray(
                    np.stack(
                        [z_all[j, h * HALF : (h + 1) * HALF, :] for (j, h) in tasks],
                        axis=1,
                    ).transpose(2, 1, 0)
                ).astype(np.float16),
            }
        )

    res = run_bass_kernel_spmd(nc, in_maps, list(range(NCORES)), **_RUN_KWARGS)
    _LAST_RESULT = res

    full = np.zeros((N, N, B, D, D), dtype=np.float32)
    for c in range(NCORES):
        i, tasks = metas[c]
        o = res.results[c]["out"]  # [NTASK, HALF, PACKED] fp16
        for t, (j, h) in enumerate(tasks):
            mirrored = np.take(o[t], _UNPACK_IDX, axis=-1)  # [HALF, D*D] fp16
            full[i, j, h * HALF : (h + 1) * HALF] = mirrored.reshape(
                HALF, D, D
            ).astype(np.float32)
    return full
